# revision 12
# baseline (speedup 1.0000x reference)
"""GAT-style attention head, distributed across 8 TRN2 NeuronCores.

Math (per batch b):
    S   = seq @ Wf                     [N, D]
    F1  = S @ w1 + b1                  [N]
    F2  = S @ w2 + b2                  [N]
    t   = F1[:, None] + F2[None, :]    [N, N]
    e   = exp(leaky_relu(t, 0.2)) = max(exp(t), exp(0.2 t))
    out = leaky_relu((e @ S) / rowsum(e) + bias, 0.2)

Softmax is row-shift invariant, so scale row i by exp(-0.2 F1_i):
    e'_ij = max(g_i * b_j, d_j)
with g = exp(0.8 F1), b = exp(F2), d = exp(0.2 F2).  The whole NxN
elementwise stage is ONE dual-scalar DVE tensor_scalar per [128, 512]
tile: (g_bc * b_scalar) max d_scalar.  This runs in the DVE's 2x_1p
mode (321 ns / tile) and is the pacing engine of the main loop.

Distribution: output rows (i) split across 8 cores; every core
recomputes the full S redundantly from a full bf16 copy of seq
(collectives on this stack pay a 50-100us first-call penalty).  The
host pre-transposes AND pre-rolls seq per core so each core's local
rows sit at column 0 of its X^T copy -- f1 reads a static slice of the
streamed X tiles, no separate local DMA.

Host also precomputes everything derived from the weights alone:
wfv = [Wf | Wf@w2] (bf16), v1 = Wf@w1, broadcast scalar constants and
the broadcast bias row -- the device prologue is 4 small DMAs.

Main-loop structure (per batch):
  - S-phase: 2 matmuls per 128-col chunk into PSUM [S | f2], Act
    copies to SBUF sa chunks laid out [S | f2 | ones]; Act computes
    bcol/dcol = exp(f2), exp(0.2 f2) per 8-chunk group so the main
    loop starts as soon as the first X chunk lands.
  - main: DVE makes e half-tiles; PE runs 8 matmuls per chunk
    (rhs = [S | f2 | ones]) accumulating [e@S | junk | rowsum].
  - epilogue: zr = 1/rowsum (DVE), y = po*zr + bias (DVE STT, PSUM
    src), out = Prelu(y, 0.2) (Act -- same act table as Exp), DMA.
16 PSUM accumulator tiles (8 per batch) decouple the batches.
"""

import os
import sys
import numpy as np

if "/opt/trn_rl_repo" not in sys.path:
    sys.path.insert(0, "/opt/trn_rl_repo")

B, N, F, D = 2, 8192, 256, 128
CORES = 8
NL = N // CORES          # 1024 output rows per core per batch
JC = N // 128            # 64 j-chunks per batch
IT = NL // 128           # 8 i-tiles per core per batch
ALPHA = 0.2
SW = D + 1               # psum S tile: [S | f2]
W = D + 2                # sa chunk:  [S | f2 | ones]
NK = 8                   # x chunks per (b, fc): [128, 1024]
KJ = JC // NK            # 8 j-chunks per x chunk

_cache = {}


def build():
    import concourse.bass as bass
    import concourse.bacc as bacc
    import concourse.mybir as mybir
    import concourse.tile as tile

    f32 = mybir.dt.float32
    bf16 = mybir.dt.bfloat16
    AF = mybir.ActivationFunctionType
    ALU = mybir.AluOpType

    nc = bacc.Bacc(None, debug=False, num_devices=CORES)

    seqf_ext = nc.declare_dram_parameter("seqfT", [B, F, N], bf16, isOutput=False)
    wfv_ext = nc.declare_dram_parameter("wfv", [128, 2, SW], bf16, isOutput=False)
    v1c_ext = nc.declare_dram_parameter("v1c", [128, 2], bf16, isOutput=False)
    consts_ext = nc.declare_dram_parameter("consts", [128, 4], f32, isOutput=False)
    biasbc_ext = nc.declare_dram_parameter("biasbc", [128, D], f32, isOutput=False)
    out_ext = nc.declare_dram_parameter("out", [B, NL, D], f32, isOutput=True)
    DBG = bool(int(os.environ.get("KERNEL_DEBUG_DUMP", "0")))
    if DBG:
        dbg_sa = nc.declare_dram_parameter("dbg_sa", [128, JC * W], bf16,
                                           isOutput=True)
        dbg_g = nc.declare_dram_parameter("dbg_g", [128, B * NL], bf16,
                                          isOutput=True)
        dbg_bd = nc.declare_dram_parameter("dbg_bd", [128, 2 * JC], f32,
                                           isOutput=True)
        dbg_f1 = nc.declare_dram_parameter("dbg_f1", [1, B * NL], f32,
                                           isOutput=True)

    with tile.TileContext(nc) as tc:
        persist_pool = tc.tile_pool(name="persist", bufs=1)
        pers = persist_pool.__enter__()

        def T(shape, dtype, name):
            return pers.tile(shape, dtype, tag=name, name=name)

        # ---------- persistent SBUF tensors ----------
        wfv = T([128, 2, SW], bf16, name="wfv")     # [Wf | Wf@w2] per f-chunk
        v1c = T([128, 2], bf16, name="v1c")         # Wf @ w1, per f-chunk
        consts = T([128, 4], f32, name="consts")
        sb1_bc = consts[:, 0:1]      # 0.8 * b1, broadcast over partitions
        b2_bc = consts[:, 1:2]       # b2
        sb2_bc = consts[:, 2:3]      # 0.2 * b2
        bias_bc = T([128, D], f32, name="bias_bc")
        ones_col = T([1, 128], f32, name="ones_col")

        xtfc = [[[T([128, 1024], bf16, name=f"xtf{b}_{fc}_{k}")
                  for k in range(NK)] for fc in range(2)] for b in range(B)]
        f1_sb = T([1, B * NL], f32, name="f1_sb")
        g_bc = T([128, B * NL], bf16, name="g_bc")  # exp(0.8 F1) bcast
        bcol = [T([128, JC], f32, name=f"bcol{b}") for b in range(B)]
        dcol = [T([128, JC], f32, name=f"dcol{b}") for b in range(B)]
        sa = [T([128, JC * W], bf16, name=f"sa{b}") for b in range(B)]
        sav = [sa[b].rearrange("p (jc w) -> p jc w", w=W) for b in range(B)]

        # ---------- small input DMAs (spread across idle rings) ----------
        nc.scalar.dma_start(out=wfv[:, :, :], in_=wfv_ext[:, :, :])
        nc.gpsimd.dma_start(out=v1c[:, :], in_=v1c_ext[:, :])
        nc.gpsimd.dma_start(out=consts[:, :], in_=consts_ext[:, :])
        nc.gpsimd.dma_start(out=bias_bc[:, :], in_=biasbc_ext[:, :])
        nc.vector.memset(ones_col[:, :], 1.0)
        for b in range(B):
            nc.vector.memset(sav[b][:, :, SW:W], 1.0)

        # ---------- X^T stream (sync HWDGE ring, in arrival order) ----------
        for b in range(B):
            for k in range(NK):
                for fc in range(2):
                    nc.sync.dma_start(
                        out=xtfc[b][fc][k][:, :],
                        in_=seqf_ext[b, fc * 128:(fc + 1) * 128,
                                     k * 1024:(k + 1) * 1024],
                    )

        with (
            tc.tile_pool(name="e_pool", bufs=4) as e_pool,
            tc.tile_pool(name="o_pool", bufs=4) as o_pool,
            tc.tile_pool(name="ph_psum", bufs=1, space="PSUM") as php,
            tc.tile_pool(name="mm_psum", bufs=1, space="PSUM") as pmm,
        ):
            # PSUM is 8 banks of 2KB/partition, bank-granular allocation.
            # Pack accumulators: 3 po slots (130 f32) per bank -> 3 banks
            # per batch for 8 accumulators; 3 ps slots in 1 bank; 1 shared
            # staging bank (pq) for the f1 row + g broadcast.  Total = 8.
            pom = [[pmm.tile([128, 3 if g < 2 else 2, W], f32,
                             tag=f"po{b}_{g}", bufs=1, name=f"po{b}_{g}")
                    for g in range(3)] for b in range(B)]
            ps_t = pmm.tile([128, 3, SW], f32, tag="ps", bufs=1, name="ps")
            pq = pmm.tile([128, 512], f32, tag="pq", bufs=1, name="pq")

            def po_sl(b, it):
                return pom[b][it // 3][:, it % 3, :]

            def emit_f1_g(b):
                # f1 row via v1 (local rows are cols 0:1024 of chunk 0 after
                # the host roll); g = exp(0.8 f1 + 0.8 b1) broadcast.  All
                # stages share the single pq staging bank (range deps
                # serialize them; only the ramp pays the chain).
                for seg in range(2):
                    sl = slice(b * NL + seg * 512, b * NL + (seg + 1) * 512)
                    for fc in range(2):
                        nc.tensor.matmul(
                            pq[0:1, :],
                            lhsT=v1c[:, fc:fc + 1],
                            rhs=xtfc[b][fc][0][:, seg * 512:(seg + 1) * 512],
                            start=(fc == 0),
                            stop=(fc == 1),
                        )
                    # DVE is idle during ramp; keep Act free for g/copies
                    nc.vector.tensor_copy(f1_sb[:, sl], pq[0:1, :])
                for seg in range(2):
                    sl = slice(b * NL + seg * 512, b * NL + (seg + 1) * 512)
                    nc.tensor.matmul(
                        pq[:, :], lhsT=ones_col[:, :], rhs=f1_sb[:, sl],
                    )
                    nc.scalar.activation(
                        g_bc[:, sl], pq[:, :], AF.Exp, bias=sb1_bc, scale=0.8,
                    )

            def emit_s_group(b, k):
                # S (+f2 column) for j-chunks [k*KJ, (k+1)*KJ) into sa, then
                # the b/d per-partition scalars for this group.
                for j in range(KJ):
                    jc = k * KJ + j
                    ps = ps_t[:, (b * JC + jc) % 3, :]
                    for fc in range(2):
                        nc.tensor.matmul(
                            ps,
                            lhsT=xtfc[b][fc][k][:, j * 128:(j + 1) * 128],
                            rhs=wfv[:, fc, :],
                            start=(fc == 0),
                            stop=(fc == 1),
                        )
                    if b == 0 and k == 0 and j >= KJ // 2:
                        # ramp only: split the first group's copies with the
                        # idle DVE so bcol group 0 is ready sooner
                        nc.vector.tensor_copy(sav[b][:, jc, 0:SW], ps)
                    else:
                        nc.scalar.copy(out=sav[b][:, jc, 0:SW], in_=ps)
                jsl = slice(k * KJ, (k + 1) * KJ)
                nc.scalar.activation(bcol[b][:, jsl], sav[b][:, jsl, D],
                                     AF.Exp, bias=b2_bc, scale=1.0)
                nc.scalar.activation(dcol[b][:, jsl], sav[b][:, jsl, D],
                                     AF.Exp, bias=sb2_bc, scale=ALPHA)

            def emit_main_group(b, k):
                for j in range(KJ):
                    jc = k * KJ + j
                    eh = [e_pool.tile([128, 512], bf16, tag=f"e{h}",
                                      name=f"e{h}", bufs=8)
                          for h in range(2)]
                    for h in range(2):
                        nc.vector.tensor_scalar(
                            out=eh[h][:, :],
                            in0=g_bc[:, b * NL + h * 512:
                                     b * NL + (h + 1) * 512],
                            scalar1=bcol[b][:, jc:jc + 1],
                            scalar2=dcol[b][:, jc:jc + 1],
                            op0=ALU.mult,
                            op1=ALU.max,
                        )
                    for it in range(IT):
                        # start=True zeroes the WHOLE PSUM bank, so only the
                        # first slot per packed bank may use it; the other
                        # slots accumulate onto the freshly-zeroed bank.
                        nc.tensor.matmul(
                            po_sl(b, it),
                            lhsT=eh[it // 4][:, (it % 4) * 128:
                                             (it % 4 + 1) * 128],
                            rhs=sa[b][:, jc * W:(jc + 1) * W],
                            start=(jc == 0 and it % 3 == 0),
                            stop=(jc == JC - 1),
                            skip_group_check=True,
                        )

            def emit_epilogue(b):
                for it in range(IT):
                    p = po_sl(b, it)
                    zr = o_pool.tile([128, 1], f32, tag="zr")
                    nc.vector.reciprocal(zr[:, :], p[:, SW:W])
                    y = o_pool.tile([128, D], f32, tag="y")
                    nc.vector.scalar_tensor_tensor(
                        out=y[:, :],
                        in0=p[:, 0:D],
                        scalar=zr[:, 0:1],
                        in1=bias_bc[:, :],
                        op0=ALU.mult,
                        op1=ALU.add,
                    )
                    o = o_pool.tile([128, D], f32, tag="o")
                    nc.vector.scalar_tensor_tensor(
                        out=o[:, :],
                        in0=y[:, :],
                        scalar=ALPHA,
                        in1=y[:, :],
                        op0=ALU.mult,
                        op1=ALU.max,
                    )
                    eng = nc.sync if it % 2 == 0 else nc.scalar
                    eng.dma_start(
                        out=out_ext[b, it * 128:(it + 1) * 128, :],
                        in_=o[:, :],
                    )

            # ---------- schedule ----------
            # Each batch runs its own S-phase JIT inside its own main loop
            # (keeping the in-order Act queue balanced: ~26us of Act work
            # per ~42us DVE-paced window).  b1 prefetches its f1/g and
            # first two S groups late in b0's loop so the batch boundary
            # has no serial S->copy->bcol chain.
            emit_f1_g(0)
            emit_s_group(0, 0)
            for g in range(NK):
                if g + 1 < NK:
                    emit_s_group(0, g + 1)
                emit_main_group(0, g)
                if g == 4:
                    emit_f1_g(1)
                if g == 5:
                    emit_s_group(1, 0)
                if g == 6:
                    emit_s_group(1, 1)
            emit_epilogue(0)
            for g in range(NK):
                if g + 2 < NK:
                    emit_s_group(1, g + 2)
                emit_main_group(1, g)
            emit_epilogue(1)

            if DBG:
                nc.sync.dma_start(out=dbg_sa[:, :], in_=sa[0][:, :])
                nc.sync.dma_start(out=dbg_g[:, :], in_=g_bc[:, :])
                nc.sync.dma_start(out=dbg_bd[:, 0:JC], in_=bcol[0][:, :])
                nc.sync.dma_start(out=dbg_bd[:, JC:2 * JC], in_=dcol[0][:, :])
                nc.sync.dma_start(out=dbg_f1[:, :], in_=f1_sb[:, :])

        persist_pool.__exit__(None, None, None)

    nc.compile()
    return nc


def _get_nc():
    if "nc" not in _cache:
        _cache["nc"] = build()
    return _cache["nc"]


def kernel(seq, Wf, w1, b1, w2, b2, bias):
    import ml_dtypes
    from concourse.bass_utils import run_bass_kernel_spmd

    bf = ml_dtypes.bfloat16
    seq = np.asarray(seq, dtype=np.float32)
    seqfT = seq.astype(bf).transpose(0, 2, 1)          # [B, F, N]
    Wf = np.asarray(Wf, dtype=np.float32)
    w1 = np.asarray(w1, dtype=np.float32).reshape(D, 1)
    b1 = np.asarray(b1, dtype=np.float32).reshape(1)
    w2 = np.asarray(w2, dtype=np.float32).reshape(D, 1)
    b2 = np.asarray(b2, dtype=np.float32).reshape(1)
    bias = np.asarray(bias, dtype=np.float32).reshape(D)

    # host-precomputed weight products / broadcasts
    v1 = (Wf @ w1)[:, 0]                               # [F]
    v2 = (Wf @ w2)[:, 0]                               # [F]
    wfv_np = np.zeros((128, 2, SW), dtype=np.float32)
    v1c_np = np.zeros((128, 2), dtype=np.float32)
    for fc in range(2):
        rows = slice(fc * 128, (fc + 1) * 128)
        wfv_np[:, fc, 0:D] = Wf[rows, :]
        wfv_np[:, fc, D] = v2[rows]
        v1c_np[:, fc] = v1[rows]
    wfv_np = np.ascontiguousarray(wfv_np.astype(bf))
    v1c_np = np.ascontiguousarray(v1c_np.astype(bf))
    consts_np = np.zeros((128, 4), dtype=np.float32)
    consts_np[:, 0] = 0.8 * b1[0]
    consts_np[:, 1] = b2[0]
    consts_np[:, 2] = ALPHA * b2[0]
    biasbc_np = np.ascontiguousarray(
        np.broadcast_to(bias[None, :], (128, D)).astype(np.float32))

    nc = _get_nc()
    in_maps = []
    for r in range(CORES):
        # roll so core r's local rows are columns 0:NL of every f-row
        sh = np.ascontiguousarray(np.roll(seqfT, -r * NL, axis=2))
        in_maps.append({
            "seqfT": sh,
            "wfv": wfv_np, "v1c": v1c_np,
            "consts": consts_np, "biasbc": biasbc_np,
        })

    trace = bool(int(os.environ.get("KERNEL_TRACE", "0")))
    if trace:
        import concourse.bass_utils as bu
        bu.upload_artifacts = lambda tmpdir: ""  # no network in container

    res = run_bass_kernel_spmd(
        nc, in_maps, core_ids=list(range(CORES)), trace=trace
    )
    _cache["last_result"] = res
    _cache["exec_time_ns"] = res.exec_time_ns

    out = np.concatenate(
        [res.results[r]["out"] for r in range(CORES)], axis=1
    )
    return np.ascontiguousarray(out.astype(np.float32))


# revision 21
# speedup vs baseline: 1.1059x; 1.1059x over previous
"""GAT-style attention head, distributed across 8 TRN2 NeuronCores.

Math (per batch b):
    S   = seq @ Wf                     [N, D]
    F1  = S @ w1 + b1                  [N]
    F2  = S @ w2 + b2                  [N]
    t   = F1[:, None] + F2[None, :]    [N, N]
    e   = exp(leaky_relu(t, 0.2)) = max(exp(t), exp(0.2 t))
    out = leaky_relu((e @ S) / rowsum(e) + bias, 0.2)

Softmax is row-shift invariant, so scale row i by exp(-0.2 F1_i):
    e'_ij = max(g_i * b_j, d_j)
with g = exp(0.8 F1), b = exp(F2), d = exp(0.2 F2).  The whole NxN
elementwise stage is ONE dual-scalar DVE tensor_scalar per [128, 512]
tile: (g_bc * b_scalar) max d_scalar.  This runs in the DVE's 2x_1p
mode (321 ns / tile) and is the pacing engine of the main loop.

Distribution: output rows (i) split across 8 cores; every core
recomputes the full S redundantly from a full bf16 copy of seq
(collectives on this stack pay a 50-100us first-call penalty).  The
host pre-transposes AND pre-rolls seq per core so each core's local
rows sit at column 0 of its X^T copy -- f1 reads a static slice of the
streamed X tiles, no separate local DMA.

Host also precomputes everything derived from the weights alone:
wfv = [Wf | Wf@w2] (bf16), v1 = Wf@w1, broadcast scalar constants and
the broadcast bias row -- the device prologue is 4 small DMAs.

Main-loop structure (per batch):
  - S-phase: 2 matmuls per 128-col chunk into PSUM [S | f2], Act
    copies to SBUF sa chunks laid out [S | f2 | ones]; Act computes
    bcol/dcol = exp(f2), exp(0.2 f2) per 8-chunk group so the main
    loop starts as soon as the first X chunk lands.
  - main: DVE makes e half-tiles; PE runs 8 matmuls per chunk
    (rhs = [S | f2 | ones]) accumulating [e@S | junk | rowsum].
  - epilogue: zr = 1/rowsum (DVE), y = po*zr + bias (DVE STT, PSUM
    src), out = Prelu(y, 0.2) (Act -- same act table as Exp), DMA.
16 PSUM accumulator tiles (8 per batch) decouple the batches.
"""

import os
import sys
import numpy as np

if "/opt/trn_rl_repo" not in sys.path:
    sys.path.insert(0, "/opt/trn_rl_repo")

B, N, F, D = 2, 8192, 256, 128
CORES = 8
NL = N // CORES          # 1024 output rows per core per batch
JC = N // 128            # 64 j-chunks per batch
IT = NL // 128           # 8 i-tiles per core per batch
ALPHA = 0.2
SW = D + 1               # psum S tile: [S | f2]
W = D + 2                # sa chunk:  [S | f2 | ones]
NK = 8                   # x chunks per (b, fc): [128, 1024]
KJ = JC // NK            # 8 j-chunks per x chunk

_cache = {}


def build():
    import concourse.bass as bass
    import concourse.bacc as bacc
    import concourse.mybir as mybir
    import concourse.tile as tile

    f32 = mybir.dt.float32
    bf16 = mybir.dt.bfloat16
    AF = mybir.ActivationFunctionType
    ALU = mybir.AluOpType

    nc = bacc.Bacc(None, debug=False, num_devices=CORES)

    seqf_ext = nc.declare_dram_parameter("seqfT", [B, F, N], bf16, isOutput=False)
    wfv_ext = nc.declare_dram_parameter("wfv", [128, 2, SW + 1], bf16,
                                        isOutput=False)
    consts_ext = nc.declare_dram_parameter("consts", [128, 4], f32, isOutput=False)
    biasbc_ext = nc.declare_dram_parameter("biasbc", [128, D], f32, isOutput=False)
    out_ext = nc.declare_dram_parameter("out", [B, NL, D], f32, isOutput=True)
    DBG = bool(int(os.environ.get("KERNEL_DEBUG_DUMP", "0")))
    if DBG:
        dbg_sa = nc.declare_dram_parameter("dbg_sa", [128, JC * W], bf16,
                                           isOutput=True)
        dbg_g = nc.declare_dram_parameter("dbg_g", [128, B * NL], bf16,
                                          isOutput=True)
        dbg_bd = nc.declare_dram_parameter("dbg_bd", [128, 2 * JC], f32,
                                           isOutput=True)
        dbg_f1 = nc.declare_dram_parameter("dbg_f1", [1, B * NL], bf16,
                                           isOutput=True)

    with tile.TileContext(nc) as tc:
        persist_pool = tc.tile_pool(name="persist", bufs=1)
        pers = persist_pool.__enter__()

        def T(shape, dtype, name):
            return pers.tile(shape, dtype, tag=name, name=name)

        # ---------- persistent SBUF tensors ----------
        wfv = T([128, 2, SW + 1], bf16, name="wfv")  # [Wf | Wf@w2 | Wf@w1]
        consts = T([128, 4], f32, name="consts")
        sb1_bc = consts[:, 0:1]      # 0.8 * b1, broadcast over partitions
        b2_bc = consts[:, 1:2]       # b2
        sb2_bc = consts[:, 2:3]      # 0.2 * b2
        bias_bc = T([128, D], f32, name="bias_bc")

        xtfc = [[[T([128, 1024], bf16, name=f"xtf{b}_{fc}_{k}")
                  for k in range(NK)] for fc in range(2)] for b in range(B)]
        g_row = T([1, B * NL], bf16, name="g_row")
        g_bc = T([128, B * NL], bf16, name="g_bc")  # exp(0.8 F1) bcast
        bcol = [T([128, JC], f32, name=f"bcol{b}") for b in range(B)]
        dcol = [T([128, JC], f32, name=f"dcol{b}") for b in range(B)]
        sa = [T([128, JC * W], bf16, name=f"sa{b}") for b in range(B)]
        sav = [sa[b].rearrange("p (jc w) -> p jc w", w=W) for b in range(B)]

        # ---------- small input DMAs (spread across idle rings) ----------
        nc.scalar.dma_start(out=wfv[:, :, :], in_=wfv_ext[:, :, :])
        nc.gpsimd.dma_start(out=consts[:, :], in_=consts_ext[:, :])
        nc.gpsimd.dma_start(out=bias_bc[:, :], in_=biasbc_ext[:, :])
        for b in range(B):
            nc.vector.memset(sav[b][:, :, SW:W], 1.0)

        # ---------- X^T stream (sync HWDGE ring, in arrival order) ----------
        for b in range(B):
            for k in range(NK):
                for fc in range(2):
                    nc.sync.dma_start(
                        out=xtfc[b][fc][k][:, :],
                        in_=seqf_ext[b, fc * 128:(fc + 1) * 128,
                                     k * 1024:(k + 1) * 1024],
                    )

        with (
            tc.tile_pool(name="e_pool", bufs=4) as e_pool,
            tc.tile_pool(name="o_pool", bufs=4) as o_pool,
            tc.tile_pool(name="ph_psum", bufs=1, space="PSUM") as php,
            tc.tile_pool(name="mm_psum", bufs=1, space="PSUM") as pmm,
        ):
            # PSUM is 8 banks of 2KB/partition, bank-granular allocation.
            # Pack accumulators: 3 po slots (130 f32) per bank -> 3 banks
            # per batch for 8 accumulators; 3 ps slots in 1 bank; 1 shared
            # staging bank (pq) for the f1 row + g broadcast.  Total = 8.
            pom = [[pmm.tile([128, 3 if g < 2 else 2, W], f32,
                             tag=f"po{b}_{g}", bufs=1, name=f"po{b}_{g}")
                    for g in range(3)] for b in range(B)]
            ps_t = pmm.tile([128, 3, SW], f32, tag="ps", bufs=1, name="ps")
            pq = pmm.tile([128, 512], f32, tag="pq", bufs=1, name="pq")

            def po_sl(b, it):
                return pom[b][it // 3][:, it % 3, :]

            def emit_f1_g(b):
                # f1 row via the v1 column of wfv (local rows are cols 0:1024
                # of chunk 0 after the host roll); g = exp(0.8 f1 + 0.8 b1)
                # via Act row-exp + gpsimd partition broadcast.
                for seg in range(2):
                    sl = slice(b * NL + seg * 512, b * NL + (seg + 1) * 512)
                    for fc in range(2):
                        nc.tensor.matmul(
                            pq[0:1, :],
                            lhsT=wfv[:, fc, SW:SW + 1],
                            rhs=xtfc[b][fc][0][:, seg * 512:(seg + 1) * 512],
                            start=(fc == 0),
                            stop=(fc == 1),
                        )
                    nc.scalar.activation(
                        g_row[:, sl], pq[0:1, :], AF.Exp,
                        bias=consts[0:1, 0:1], scale=0.8,
                    )
                    nc.gpsimd.partition_broadcast(g_bc[:, sl], g_row[:, sl])

            def emit_s_group(b, k):
                # S (+f2 column) for j-chunks [k*KJ, (k+1)*KJ) into sa, then
                # the b/d per-partition scalars for this group.  Copies are
                # paired (2 chunks per Act op) when their ps slots are
                # adjacent in the packed bank.
                ramp = (b == 0 and k == 0)
                held = None
                for j in range(KJ):
                    jc = k * KJ + j
                    slot = (b * JC + jc) % 3
                    for fc in range(2):
                        nc.tensor.matmul(
                            ps_t[:, slot, :],
                            lhsT=xtfc[b][fc][k][:, j * 128:(j + 1) * 128],
                            rhs=wfv[:, fc, 0:SW],
                            start=(fc == 0),
                            stop=(fc == 1),
                        )
                    if ramp:
                        # singles split Act/DVE so bcol group 0 lands ASAP
                        if j >= KJ // 2:
                            nc.vector.tensor_copy(sav[b][:, jc, 0:SW],
                                                  ps_t[:, slot, :])
                        else:
                            nc.scalar.copy(out=sav[b][:, jc, 0:SW],
                                           in_=ps_t[:, slot, :])
                        continue
                    if slot == 0:
                        held = jc
                    elif slot == 1 and held == jc - 1:
                        nc.scalar.copy(out=sav[b][:, jc - 1:jc + 1, 0:SW],
                                       in_=ps_t[:, 0:2, :])
                        held = None
                    else:
                        nc.scalar.copy(out=sav[b][:, jc, 0:SW],
                                       in_=ps_t[:, slot, :])
                if held is not None:
                    nc.scalar.copy(out=sav[b][:, held, 0:SW],
                                   in_=ps_t[:, 0, :])
                jsl = slice(k * KJ, (k + 1) * KJ)
                nc.scalar.activation(bcol[b][:, jsl], sav[b][:, jsl, D],
                                     AF.Exp, bias=b2_bc, scale=1.0)
                nc.scalar.activation(dcol[b][:, jsl], sav[b][:, jsl, D],
                                     AF.Exp, bias=sb2_bc, scale=ALPHA)

            def emit_main_group(b, k):
                for j in range(KJ):
                    jc = k * KJ + j
                    eh = [e_pool.tile([128, 512], bf16, tag=f"e{h}",
                                      name=f"e{h}", bufs=8)
                          for h in range(2)]
                    for h in range(2):
                        nc.vector.tensor_scalar(
                            out=eh[h][:, :],
                            in0=g_bc[:, b * NL + h * 512:
                                     b * NL + (h + 1) * 512],
                            scalar1=bcol[b][:, jc:jc + 1],
                            scalar2=dcol[b][:, jc:jc + 1],
                            op0=ALU.mult,
                            op1=ALU.max,
                        )
                    for it in range(IT):
                        # start=True zeroes the WHOLE PSUM bank, so only the
                        # first slot per packed bank may use it; the other
                        # slots accumulate onto the freshly-zeroed bank.
                        nc.tensor.matmul(
                            po_sl(b, it),
                            lhsT=eh[it // 4][:, (it % 4) * 128:
                                             (it % 4 + 1) * 128],
                            rhs=sa[b][:, jc * W:(jc + 1) * W],
                            start=(jc == 0 and it % 3 == 0),
                            stop=(jc == JC - 1),
                            skip_group_check=True,
                        )

            def emit_epilogue(b):
                for it in range(IT):
                    p = po_sl(b, it)
                    zr = o_pool.tile([128, 1], f32, tag="zr")
                    nc.vector.reciprocal(zr[:, :], p[:, SW:W])
                    y = o_pool.tile([128, D], f32, tag="y")
                    nc.vector.scalar_tensor_tensor(
                        out=y[:, :],
                        in0=p[:, 0:D],
                        scalar=zr[:, 0:1],
                        in1=bias_bc[:, :],
                        op0=ALU.mult,
                        op1=ALU.add,
                    )
                    o = o_pool.tile([128, D], f32, tag="o")
                    nc.vector.scalar_tensor_tensor(
                        out=o[:, :],
                        in0=y[:, :],
                        scalar=ALPHA,
                        in1=y[:, :],
                        op0=ALU.mult,
                        op1=ALU.max,
                    )
                    # sync ring is idle after the X stream; keep Act free
                    # for the other batch's S-copies
                    nc.sync.dma_start(
                        out=out_ext[b, it * 128:(it + 1) * 128, :],
                        in_=o[:, :],
                    )

            # ---------- schedule ----------
            # Each batch runs its own S-phase JIT inside its own main loop
            # (keeping the in-order Act queue balanced: ~26us of Act work
            # per ~42us DVE-paced window).  b1 prefetches its f1/g and
            # first two S groups late in b0's loop so the batch boundary
            # has no serial S->copy->bcol chain.
            emit_f1_g(0)
            emit_s_group(0, 0)
            for g in range(NK):
                if g + 1 < NK:
                    emit_s_group(0, g + 1)
                emit_main_group(0, g)
                if g == 4:
                    emit_f1_g(1)
                if g == 5:
                    emit_s_group(1, 0)
                if g == 6:
                    emit_s_group(1, 1)
            emit_epilogue(0)
            for g in range(NK):
                if g + 2 < NK:
                    emit_s_group(1, g + 2)
                emit_main_group(1, g)
            emit_epilogue(1)

            if DBG:
                nc.sync.dma_start(out=dbg_sa[:, :], in_=sa[0][:, :])
                nc.sync.dma_start(out=dbg_g[:, :], in_=g_bc[:, :])
                nc.sync.dma_start(out=dbg_bd[:, 0:JC], in_=bcol[0][:, :])
                nc.sync.dma_start(out=dbg_bd[:, JC:2 * JC], in_=dcol[0][:, :])
                nc.sync.dma_start(out=dbg_f1[:, :], in_=g_row[:, :])

        persist_pool.__exit__(None, None, None)

    nc.compile()
    return nc


def _get_nc():
    if "nc" not in _cache:
        _cache["nc"] = build()
    return _cache["nc"]


def kernel(seq, Wf, w1, b1, w2, b2, bias):
    import ml_dtypes
    from concourse.bass_utils import run_bass_kernel_spmd

    bf = ml_dtypes.bfloat16
    seq = np.asarray(seq, dtype=np.float32)
    seqfT = seq.astype(bf).transpose(0, 2, 1)          # [B, F, N]
    Wf = np.asarray(Wf, dtype=np.float32)
    w1 = np.asarray(w1, dtype=np.float32).reshape(D, 1)
    b1 = np.asarray(b1, dtype=np.float32).reshape(1)
    w2 = np.asarray(w2, dtype=np.float32).reshape(D, 1)
    b2 = np.asarray(b2, dtype=np.float32).reshape(1)
    bias = np.asarray(bias, dtype=np.float32).reshape(D)

    # host-precomputed weight products / broadcasts
    v1 = (Wf @ w1)[:, 0]                               # [F]
    v2 = (Wf @ w2)[:, 0]                               # [F]
    wfv_np = np.zeros((128, 2, SW + 1), dtype=np.float32)
    for fc in range(2):
        rows = slice(fc * 128, (fc + 1) * 128)
        wfv_np[:, fc, 0:D] = Wf[rows, :]
        wfv_np[:, fc, D] = v2[rows]
        wfv_np[:, fc, SW] = v1[rows]
    wfv_np = np.ascontiguousarray(wfv_np.astype(bf))
    consts_np = np.zeros((128, 4), dtype=np.float32)
    consts_np[:, 0] = 0.8 * b1[0]
    consts_np[:, 1] = b2[0]
    consts_np[:, 2] = ALPHA * b2[0]
    biasbc_np = np.ascontiguousarray(
        np.broadcast_to(bias[None, :], (128, D)).astype(np.float32))

    nc = _get_nc()
    in_maps = []
    for r in range(CORES):
        # roll so core r's local rows are columns 0:NL of every f-row
        sh = np.ascontiguousarray(np.roll(seqfT, -r * NL, axis=2))
        in_maps.append({
            "seqfT": sh,
            "wfv": wfv_np,
            "consts": consts_np, "biasbc": biasbc_np,
        })

    trace = bool(int(os.environ.get("KERNEL_TRACE", "0")))
    if trace:
        import concourse.bass_utils as bu
        bu.upload_artifacts = lambda tmpdir: ""  # no network in container

    res = run_bass_kernel_spmd(
        nc, in_maps, core_ids=list(range(CORES)), trace=trace
    )
    _cache["last_result"] = res
    _cache["exec_time_ns"] = res.exec_time_ns

    out = np.concatenate(
        [res.results[r]["out"] for r in range(CORES)], axis=1
    )
    return np.ascontiguousarray(out.astype(np.float32))
